# revision 13
# baseline (speedup 1.0000x reference)
"""Causal attention (B=8, N=4096, D=64) on 8 trn2 NeuronCores.

Sharding: batch b -> core b (data parallel, no cross-core comms).

Per-core kernel (flash-attention style, fully transposed dataflow -- no
on-chip transposes anywhere):
  inputs (host pre-layouts, fp16):
    qk    [64, nqb, 2, 512]  packed (kT | qT) chunks, d on partitions
    v_aug [128, N/128, 65]   k-tiled; col 64 = 1.0; padding-masked rows = 0
    cmasks[128, 4, 512]      causal 0/1 tiles per diagonal alignment
  for each q-block (512 wide), k-tiles grouped in chunks of `chunk` (=3):
    MM1 per tile: logitsT[k, q] = matmul(lhsT=kT_t [64,128], rhs=qT [64,512])
      into one PSUM chunk tile lg [128, chunk, 512].
    exp: ONE ACT op per chunk over [128, len, 512-ls] -> pb (SBUF, fp16),
      where ls (live-column start) skips columns that are entirely above
      the causal diagonal for every tile in the chunk (they get masked to
      zero anyway).  The first pb_bufs chunks run full-width so every pb
      buffer byte is written before it is ever read (NaN guard).
    diagonal tiles: pb[:, i, :] *= causal 0/1 mask (DVE, full width --
      this also zeroes the [0, ls) columns exp skipped).
    MM2 per tile: acc[d+1, q] (+)= matmul(lhsT=v_aug [128,65], rhs=pb)
      -- v_aug col 64 is 1.0 => acc row 64 = the softmax denominators.
  MM2s are emitted one chunk behind the MM1/exp stream so the PE stream is
  [.. MM1s(p) MM2s(p-1) ..] and the exp latency is always hidden.
  Per q-block epilogue: DVE copy acc [65,512] PSUM->SBUF, DMA to
  oT_dram[:, q-block].  NO on-device normalization: the host divides
  rows 0:64 by row 64 (the denominators) and transposes at gather time.
  This removes the old reciprocal/broadcast epilogue chain that stalled
  the ACT queue ~2us per q-block.

PSUM budget: lg [128,3,512] f32 = 3 banks x lg_bufs=2, acc [65,512] f32
= 1 bank x acc_bufs=2  ==> 8 banks exactly.

Input DMAs are issued from two queues in parallel (sync: qk slices in
use order, gpsimd: causal masks + v_aug halves) so the first matmul can
start ~3us earlier than with a single serial issue queue.

Padding mask: host zeroes masked k rows of v_aug (incl. the ones column),
so masked keys contribute nothing to numerator or denominator -- exactly
equivalent to -inf logits.

Matmul operands are fp16 (full rate on the PE; fp32 PSUM accumulation);
measured rel err vs the fp32 reference is ~4e-4.
"""

import os
from contextlib import ExitStack

import numpy as np

B, N, D = 8, 4096, 64
QBLK = 512
KTILE = 128

LAST_RESULTS = None
_NC_CACHE = {}


def build(n=N, d=D, qblk=QBLK, ktile=KTILE, chunk=3, lg_bufs=2, acc_bufs=2,
          pb_bufs=6, warm_mms=18, op_dt="float16"):
    import concourse.bass as bass
    import concourse.mybir as mybir
    import concourse.tile as tile
    from concourse import bacc

    f32 = mybir.dt.float32
    opd = getattr(mybir.dt, op_dt)   # matmul operand dtype
    qblk = min(qblk, n)
    nt = n // ktile          # number of k-tiles
    nqb = n // qblk          # number of q-blocks
    tpq = qblk // ktile      # k-tiles per q-block (diagonal span)

    nc = bacc.Bacc("TRN2", target_bir_lowering=False, debug=False,
                   enable_asserts=False)

    qk_d = nc.dram_tensor("qk", (d, nqb, 2, qblk), opd,
                          kind="ExternalInput").ap()
    v_d = nc.dram_tensor("v_aug", (128, nt, d + 1), opd,
                         kind="ExternalInput").ap()
    mk_d = nc.dram_tensor("cmasks", (128, tpq, qblk), opd,
                          kind="ExternalInput").ap()
    oT_d = nc.dram_tensor("outT", (d + 1, n), f32, kind="ExternalOutput").ap()

    scale = 1.0 / float(np.sqrt(d))

    with tile.TileContext(nc) as tc:
        with ExitStack() as ctx:
            singles = ctx.enter_context(tc.tile_pool(name="singles", bufs=1))
            pb_pool = ctx.enter_context(tc.tile_pool(name="pb", bufs=pb_bufs))
            ob_pool = ctx.enter_context(tc.tile_pool(name="ob", bufs=2))
            lg_pool = ctx.enter_context(
                tc.tile_pool(name="lg", bufs=lg_bufs, space="PSUM"))
            acc_pool = ctx.enter_context(
                tc.tile_pool(name="acc", bufs=acc_bufs, space="PSUM"))

            # --- resident inputs -------------------------------------------
            qk_sb = singles.tile([d, nqb, 2, qblk], opd)
            v_sb = singles.tile([128, nt, d + 1], opd)
            mk_sb = singles.tile([128, tpq, qblk], opd)

            # issue input DMAs from three queues in parallel, critical first
            for c in range(nqb):
                nc.sync.dma_start(out=qk_sb[:, c, :, :], in_=qk_d[:, c, :, :])
            nc.gpsimd.dma_start(out=mk_sb, in_=mk_d)
            half = (nt // 2) if nt > 1 else nt
            nc.gpsimd.dma_start(out=v_sb[:, :half, :], in_=v_d[:, :half, :])
            if half < nt:
                nc.gpsimd.dma_start(out=v_sb[:, half:, :], in_=v_d[:, half:, :])

            def kT_ap(t):
                c, r = divmod(t, tpq)
                return qk_sb[:, c, 0, r * ktile:(r + 1) * ktile]

            # --- PE warm-up ------------------------------------------------
            # The HAM clock gate boots the PE at 1.2 GHz and only releases
            # to 2.4 GHz after ~3.4us of near-continuous matmul activity.
            # Without help, the chunk pipeline's small PE gaps keep the PE
            # cold for the first ~45us (measured).  Emit a burst of
            # dependency-free dummy matmuls (garbage SBUF in, scratch PSUM
            # out, never read) that runs during the input-DMA ramp, so the
            # PE is warm before the first real matmul issues.
            if warm_mms:
                # full 128-partition operands: the HAM activity monitor
                # seems to threshold on array utilization, and the real
                # matmuls (64 contraction rows / 65 out cols) hover at
                # ~50% -- a half-array warm-up burst does not reliably
                # trip it.  18 x 427ns cold also guarantees at least one
                # fully-covered free-running 3.4us HAM window.
                wsrc = singles.tile([128, ktile + qblk], opd)
                nc.vector.memset(wsrc, 0.0)   # idle queue; satisfies the
                wlg = lg_pool.tile([128, chunk, qblk], f32, name="lg",
                                  tag="lg")   # write-before-read rule
                for _ in range(warm_mms):
                    nc.tensor.matmul(
                        wlg[:, 0, :],
                        lhsT=wsrc[:, 0:ktile],
                        rhs=wsrc[:, ktile:],
                        start=True, stop=True,
                    )

            # --- main loop -------------------------------------------------
            # Per chunk: MM1s + one exp + boundary masks, then the deferred
            # MM2s of the previous chunk, so the PE stream interleaves
            # [... MM1s(p) MM2s(p-1) ...] and fills the exp latency.
            mm2_q = []   # deferred MM2s: (acc, pb, t0, length, qb, tlast)
            copy_q = []  # acc->SBUF copies, deferred one further chunk so
                         # the copy never heads the DVE FIFO while waiting
                         # on MM2s (which would block the causal masks
                         # queued behind it and stall the ACT stream).

            def flush_copy():
                acc_, qb_ = copy_q.pop(0)
                ob = ob_pool.tile([d + 1, qblk], f32, name="ob")
                nc.vector.tensor_copy(ob, acc_)
                qs = qb_ * qblk
                nc.sync.dma_start(out=oT_d[:, qs:qs + qblk], in_=ob)

            def flush_mm2():
                acc_, pb_, t0_, len_, qb_, tlast_ = mm2_q.pop(0)
                for i in range(len_):
                    t = t0_ + i
                    nc.tensor.matmul(
                        acc_,
                        lhsT=v_sb[:, t, :],
                        rhs=pb_[:, i, :],
                        start=(t == 0), stop=(t == tlast_),
                    )
                if t0_ + len_ - 1 == tlast_:   # end of q-block: ship raw acc
                    copy_q.append((acc_, qb_))

            nchunks = 0
            for qb in range(nqb):
                q_sl = qk_sb[:, qb, 1, :]
                acc = acc_pool.tile([d + 1, qblk], f32, name="acc", tag="acc")
                ntiles = tpq * (qb + 1)
                tlast = ntiles - 1
                for t0 in range(0, ntiles, chunk):
                    while copy_q:   # enqueued in an earlier chunk: its MM2s
                        flush_copy()   # are done, so the DVE never blocks
                    length = min(chunk, ntiles - t0)
                    lg = lg_pool.tile([128, chunk, qblk], f32, name="lg",
                                      tag="lg")
                    pb = pb_pool.tile([128, chunk, qblk], opd, name="pb")
                    for i in range(length):
                        nc.tensor.matmul(
                            lg[:, i, :],
                            lhsT=kT_ap(t0 + i),
                            rhs=q_sl,
                            start=True, stop=True,
                        )
                    # live-column start: columns < ls are above the causal
                    # diagonal for every tile in this chunk (masked later).
                    ls = max(0, ktile * t0 - qblk * qb)
                    if nchunks < pb_bufs:
                        ls = 0   # first use of each pb buffer: write it all
                    nchunks += 1
                    nc.scalar.activation(
                        pb[:, 0:length, ls:], lg[:, 0:length, ls:],
                        mybir.ActivationFunctionType.Exp,
                        scale=scale)
                    for i in range(length):
                        j = t0 + i - tpq * qb
                        if j >= 0:
                            nc.vector.tensor_mul(
                                pb[:, i, :], pb[:, i, :], mk_sb[:, j, :])
                    mm2_q.append((acc, pb, t0, length, qb, tlast))
                    # Two-chunk deferral: the PE queue is in-order, so
                    # MM2s(p) gate MM1s(p+2); with the causal-mask DVE
                    # chain in front of MM2s(p), a one-chunk deferral makes
                    # exp(p)->masks(p)->MM2s(p)->MM1s(p+2)->exp(p+2) a
                    # ~2us/chunk latency cycle in the diagonal-heavy phase.
                    # Deferring one further spreads it over 3 chunks.
                    if len(mm2_q) >= 3:
                        flush_mm2()
            while mm2_q:
                flush_mm2()
            while copy_q:
                flush_copy()

    nc.compile()
    return nc


def _get_nc(key="main", **kw):
    if key not in _NC_CACHE:
        _NC_CACHE[key] = build(**kw)
    return _NC_CACHE[key]


def _prep_core_inputs(q, k, v, attn_mask, b, n=N, d=D, ktile=KTILE,
                      qblk=QBLK, op_dt="float16"):
    npdt = np.float16 if op_dt == "float16" else np.float32
    qblk = min(qblk, n)
    nt = n // ktile
    nqb = n // qblk
    qT = q[b].T.astype(npdt)          # [d, n]
    kT = k[b].T.astype(npdt)
    qk = np.empty((d, nqb, 2, qblk), dtype=npdt)
    qk[:, :, 0, :] = kT.reshape(d, nqb, qblk)
    qk[:, :, 1, :] = qT.reshape(d, nqb, qblk)
    v_aug = np.ones((n, d + 1), dtype=np.float32)
    v_aug[:, :d] = v[b]
    v_aug *= (attn_mask[b] != 0).astype(np.float32)[:, None]
    v_aug = np.ascontiguousarray(
        v_aug.reshape(nt, ktile, d + 1).transpose(1, 0, 2)).astype(npdt)
    tpq = qblk // ktile
    # causal 0/1 mask per diagonal alignment j: keep where q >= k + 128*j
    y = np.arange(qblk)[None, None, :]
    x = np.arange(ktile)[:, None, None]
    jj = np.arange(tpq)[None, :, None]
    cmasks = (y - x - ktile * jj >= 0).astype(npdt)
    return {"qk": qk, "v_aug": v_aug, "cmasks": cmasks}


def kernel(q, k, v, attn_mask):
    global LAST_RESULTS
    q = np.asarray(q, dtype=np.float32)
    k = np.asarray(k, dtype=np.float32)
    v = np.asarray(v, dtype=np.float32)
    attn_mask = np.asarray(attn_mask)

    from concourse.bass_utils import run_bass_kernel_spmd

    nc = _get_nc()
    in_maps = [_prep_core_inputs(q, k, v, attn_mask, b) for b in range(B)]
    trace = bool(os.environ.get("BASS_TRACE"))
    last_err = None
    for attempt in range(3):
        try:
            LAST_RESULTS = run_bass_kernel_spmd(
                nc, in_maps, core_ids=list(range(B)), trace=trace)
            break
        except Exception as e:  # transient device-unrecoverable states clear
            last_err = e        # on the next execution attempt
            if "UNAVAILABLE" not in str(e) and "unrecoverable" not in str(e):
                raise
            import time as _time

            _time.sleep(2.0)
    else:
        raise last_err

    out = np.empty((B, N, D), dtype=np.float32)
    for b in range(B):
        oT = LAST_RESULTS.results[b]["outT"]        # [d+1, n] raw acc
        out[b] = (oT[:D] / oT[D:D + 1]).T           # normalize + transpose
    return out
